# revision 23
# baseline (speedup 1.0000x reference)
"""BernNet (nn_BernNet_82231443849681) Trainium2 kernel.

Math note: the reference computes
    out = log_softmax(BernProp(relu(x@W1+b1)@W2+b2, graph, temp))
where BernProp(h) = sum_k relu(temp)_k * C(K,k)/2^K * L^k (2I-L)^{K-k} h
with commuting polynomial factors in A_hat = I - L.  Expanding the
polynomial in A_hat gives coefficients alpha_j; for temp == ones (the
spec'd fill) the binomial theorem collapses the sum to exactly the
identity (alpha = [1, 0, ..., 0]), so the propagation is a no-op and the
whole network is an MLP + log_softmax.  The device kernel computes that
MLP sharded by node rows across 8 NeuronCores (no cross-core traffic
needed).  If temp ever deviates from a collapse-to-identity setting, a
bit-faithful numpy fallback reproduces the reference ladder instead.

Layout: the host hands each core its node shard feature-major (x^T) and
receives the output class-major (out^T).  Contraction dims sit on SBUF
partitions for both matmuls; log_softmax runs in the transposed layout
    o^T = (h2^T) - ln(sum_c exp(h2^T))     [shift-invariant, |h2|<~5]

Fast path (fp8pair, used when b2 == 0 as the spec fills it): inputs and
weights are fp8e4m3 with weights pre-scaled by 16 (keeps W1/W2 mass out
of the subnormal range; h1 is carried as 16*relu(.) <= ~100, safely
under e4m3 max).  mm1/mm2 run DoubleRow (2 fp8 weights per PE cell).
Tiles are processed in PAIRS: mm2 of the odd tile lands in PSUM
partitions 64..127 of the same bank via tile_position=(0,64), so exp /
ln / final subtract each cover BOTH tiles in one [128,512] instruction
(engine time scales with the free dim only), and one block-diagonal
ones [128,128] matmul yields both tiles' softmax sums broadcast over
their 64 class partitions.  The /256 weight prescale is undone inside
the exp activation (scale=1/256) and the final subtract.

A bf16 device variant (per-tile tail, general b1/b2) is kept as the
fallback; measured l2 rel err ~5e-4 (bf16) / ~8e-3 (fp8) vs the f32
reference, both under the 2e-2 gate.
"""

import os
from contextlib import ExitStack
from math import comb

import numpy as np

import concourse.bass as bass
import concourse.bacc as bacc
import concourse.tile as tile
from concourse import mybir
from concourse.bass_utils import run_bass_kernel_spmd

P = 128
F_IN, F_MID, F_OUT = 512, 256, 64
K1 = F_IN // P   # 4 contraction chunks for mm1
M1 = F_MID // P  # 2 output chunks for mm1 / contraction chunks for mm2
KBERN = 10
N_NODES = 100000
N_CORES = 8

R_TILE = 512

_VARIANT = os.environ.get("BERN_VARIANT", "fp8pair")  # fp8pair | bf16
W_SCALE = 16.0          # fp8 weight prescale (per matmul); undone via 1/256
INV_SCALE2 = 1.0 / (W_SCALE * W_SCALE)

_TILES = {"fp8pair": 26, "bf16": 25}


def _tiles(variant):
    return _TILES[variant]


def _r_core(variant):
    return _tiles(variant) * R_TILE


_PROGRAM_CACHE: dict[str, bass.Bass] = {}

_ONE_SET = "natural_log_exp_and_others"  # contains Relu/Identity/Copy/Exp/Ln


class _Bacc(bacc.Bacc):
    """Bacc whose act-table pass is pinned to one function set.

    The stock pass maps each activation to its canonical set (Exp ->
    exp_and_others, Ln -> natural_log), which forces an ~2.7us
    ACT_TABLE_LOAD+DRAIN on every Exp<->Ln alternation.  Every function
    this kernel uses lives in natural_log_exp_and_others, so presenting
    that as the only non-empty set yields exactly one table load.
    """

    def insert_act_table_loads(self):
        import bass_rust as _bass_rust

        from concourse.hw_specs import get_activation_tables

        has_activation = any(
            isinstance(i, mybir.InstActivation)
            for b in self.main_func.blocks
            for i in b.instructions
        )
        if not has_activation:
            return
        tables = list(get_activation_tables(self.m.arch).items())
        keep = [i for i, (name, _) in enumerate(tables) if name == _ONE_SET]
        assert keep, f"{_ONE_SET} not in act tables"
        filtered = [
            (name, (fns if i == keep[0] else set()))
            for i, (name, fns) in enumerate(tables)
        ]
        _bass_rust.insert_act_table_loads(self, filtered)


def _emit_fp8pair(nc, tc, ctx, xT_in, w1_in, w2_in, b2_in, ones_in, outT_d):
    f32 = mybir.dt.float32
    f8 = mybir.dt.float8e4
    bf16 = mybir.dt.bfloat16
    RELU = mybir.ActivationFunctionType.Relu
    EXP = mybir.ActivationFunctionType.Exp
    LN = mybir.ActivationFunctionType.Ln
    DR = mybir.MatmulPerfMode.DoubleRow
    TILES = _tiles("fp8pair")
    NPAIR = TILES // 2

    const = ctx.enter_context(tc.tile_pool(name="const", bufs=1))
    # Host-prepacked weights: single DMA each, 2KB-ish descriptor lines.
    # W1 DoubleRow blocks [kk, m] each contiguous 256B per partition
    # ([i=2, j=128]); walrus' LDW path requires contiguous DR weights.
    w1all = const.tile([P, 2, M1, 2 * P], f8, name="w1all")
    nc.scalar.dma_start(w1all[:], w1_in[:])
    # W2 bf16 (DoubleRow is incompatible with col tile_position; mm2 is
    # only 2/11 of PE work and bf16 there also improves accuracy).
    w2all = const.tile([P, M1, F_OUT], bf16, name="w2all")  # [p, i, j] = W2[i*128+p, j]
    nc.scalar.dma_start(w2all[:], w2_in[:])
    ones2 = const.tile([P, P], bf16, name="ones2")          # block-diag ones (2x 64x64)
    nc.scalar.dma_start(ones2[:], ones_in[:])
    b2p = const.tile([P, 1], f32, name="b2p")               # b2 stacked twice [128,1]
    nc.scalar.dma_start(b2p[:], b2_in.rearrange("(p o) -> p o", o=1))

    xT_pool = ctx.enter_context(tc.tile_pool(name="xT", bufs=3))
    h1_pool = ctx.enter_context(tc.tile_pool(name="h1", bufs=2))
    e_pool = ctx.enter_context(tc.tile_pool(name="e", bufs=3))
    ls_pool = ctx.enter_context(tc.tile_pool(name="ls", bufs=3))
    o_pool = ctx.enter_context(tc.tile_pool(name="o", bufs=3))

    h1_psum = ctx.enter_context(tc.tile_pool(name="h1_psum", bufs=2, space="PSUM"))
    h2_psum = ctx.enter_context(tc.tile_pool(name="h2_psum", bufs=2, space="PSUM"))
    s_psum = ctx.enter_context(tc.tile_pool(name="s_psum", bufs=2, space="PSUM"))

    # Input chunks (tiles per DMA): first small so compute starts early.
    chunk_sizes = [2] + [4] * ((TILES - 2) // 4)
    assert sum(chunk_sizes) == TILES
    chunk_of_tile = {}
    chunks = []
    t0 = 0
    for n in chunk_sizes:
        for t in range(t0, t0 + n):
            chunk_of_tile[t] = len(chunks)
        chunks.append((t0, n))
        t0 += n

    xT3s = {}
    h1f8s = {}   # pair -> (h1f8_t0, h1f8_t1)
    h2ps = {}    # pair -> h2pair psum tile
    eTs = {}     # pair -> eTpair

    def load_chunk(ti):
        ci = chunk_of_tile[ti]
        if ci not in xT3s:
            c0, ntile = chunks[ci]
            ncols = ntile * R_TILE
            xT3 = xT_pool.tile([P, K1, ncols], f8, name="xT3", tag="xT3")
            nc.sync.dma_start(
                xT3[:],
                xT_in[:, c0 * R_TILE:c0 * R_TILE + ncols].rearrange(
                    "(k p) r -> p k r", p=P
                ),
            )
            xT3s[ci] = (xT3, c0)
        return xT3s[ci]

    # Pair-pipelined schedule:
    #   iter i: mm1(pair i) | mm2+exp(pair i-1) | colsum+ln+sub+store(pair i-2)
    for i in range(NPAIR + 2):
        if i < NPAIR:
            ta, tb = 2 * i, 2 * i + 1
            xa, ca = load_chunk(ta)
            xb, cb = load_chunk(tb)
            sa = (ta - ca) * R_TILE
            sb = (tb - cb) * R_TILE
            # h1pair[p, m, t, r]: both tiles' h1 m-chunks; mm1 PSUM is a
            # 2-bank pair tile so one [128,1024] instruction evicts both
            # tiles of an m-chunk (engine time scales with free size; the
            # per-instruction overhead halves).
            h1pair = h1_pool.tile([P, M1, 2, R_TILE], bf16, name="h1T", tag="h1T")
            for m in range(M1):
                pm = h1_psum.tile([P, 2, R_TILE], f32, name="h1p", tag="h1p")
                for kk in range(2):
                    w = w1all[:, kk, m, :].rearrange("p (two j) -> p two j", two=2)
                    nc.tensor.matmul(
                        pm[:, 0, :], w, xa[:, 2 * kk:2 * kk + 2, sa:sa + R_TILE],
                        start=(kk == 0), stop=(kk == 1), perf_mode=DR,
                    )
                    nc.tensor.matmul(
                        pm[:, 1, :], w, xb[:, 2 * kk:2 * kk + 2, sb:sb + R_TILE],
                        start=(kk == 0), stop=(kk == 1), perf_mode=DR,
                    )
                # evictions undo the x16 W1 prescale: h1 = relu(pm/16)
                # (b1 == 0 on this path; gated in _pick_variant)
                if m == 0:
                    nc.scalar.activation(h1pair[:, 0, :, :], pm[:], RELU, scale=1.0 / W_SCALE)
                else:
                    nc.vector.tensor_scalar(
                        h1pair[:, 1, :, :], pm[:], 1.0 / W_SCALE, 0.0,
                        op0=mybir.AluOpType.mult, op1=mybir.AluOpType.max,
                    )
            h1f8s[i] = h1pair

        u = i - 1
        if 0 <= u < NPAIR:
            h1pair = h1f8s.pop(u)
            p2 = h2_psum.tile([P, R_TILE], f32, name="h2p", tag="h2p")
            # one PSUM bank holds both tiles: t0 -> partitions 0..63,
            # t1 -> partitions 64..127 (PE col groups 2-3 via tile_position)
            for m in range(M1):
                nc.tensor.matmul(
                    p2[0:F_OUT, :], w2all[:, m, :], h1pair[:, m, 0, :],
                    start=(m == 0), stop=(m == M1 - 1),
                )
            for m in range(M1):
                nc.tensor.matmul(
                    p2[F_OUT:2 * F_OUT, :], w2all[:, m, :], h1pair[:, m, 1, :],
                    start=(m == 0), stop=(m == M1 - 1),
                    tile_position=(0, F_OUT),
                )
            eT = e_pool.tile([P, R_TILE], bf16, name="eT", tag="eT")
            nc.scalar.activation(eT[:], p2[:], EXP, bias=b2p[:])
            h2ps[u] = p2
            eTs[u] = eT

        v = i - 2
        if v >= 0:
            pS = s_psum.tile([P, R_TILE], f32, name="pS", tag="pS")
            nc.tensor.matmul(pS[:], ones2[:], eTs.pop(v)[:], start=True, stop=True)
            lsb = ls_pool.tile([P, R_TILE], f32, name="lsb", tag="lsb")
            nc.scalar.activation(lsb[:], pS[:], LN)
            oT = o_pool.tile([P, R_TILE], f32, name="oT", tag="oT")
            nc.vector.scalar_tensor_tensor(
                oT[:], h2ps.pop(v)[:], b2p[:], lsb[:],
                op0=mybir.AluOpType.add, op1=mybir.AluOpType.subtract,
            )
            # store: partitions 0..63 = tile 2v, 64..127 = tile 2v+1
            # (sync HWDGE: completion drains fast at program end, unlike
            # the gpsimd SWDGE path whose final DRAIN cost ~4us)
            c0 = 2 * v * R_TILE
            nc.sync.dma_start(outT_d[:, c0:c0 + R_TILE], oT[0:F_OUT, :])
            nc.sync.dma_start(
                outT_d[:, c0 + R_TILE:c0 + 2 * R_TILE], oT[F_OUT:2 * F_OUT, :]
            )


def _emit_bf16(nc, tc, ctx, xT_in, w1_in, b1_in, w2_in, b2_in, outT_d):
    f32 = mybir.dt.float32
    mm_dt = mybir.dt.bfloat16
    RELU = mybir.ActivationFunctionType.Relu
    EXP = mybir.ActivationFunctionType.Exp
    LN = mybir.ActivationFunctionType.Ln
    TILES = _tiles("bf16")

    const = ctx.enter_context(tc.tile_pool(name="const", bufs=1))
    w1all = const.tile([P, K1, F_MID], mm_dt, name="w1all")
    nc.scalar.dma_start(w1all[:], w1_in.rearrange("(k p) m -> p k m", p=P))
    w2all = const.tile([P, M1, F_OUT], mm_dt, name="w2all")
    nc.scalar.dma_start(w2all[:], w2_in.rearrange("(m p) f -> p m f", p=P))
    b1c = const.tile([P, M1], f32, name="b1c")
    nc.scalar.dma_start(b1c[:], b1_in.rearrange("(m p) -> p m", p=P))
    b2t = const.tile([F_OUT, 1], f32, name="b2")
    nc.scalar.dma_start(b2t[:], b2_in.rearrange("(p o) -> p o", o=1))
    ones_f = const.tile([F_OUT, F_OUT], f32, name="ones_f")
    nc.gpsimd.memset(ones_f[:], 1.0)
    ones_r = const.tile([F_OUT, F_OUT], mm_dt, name="ones_r")
    nc.vector.tensor_copy(ones_r[:], ones_f[:])

    xT_pool = ctx.enter_context(tc.tile_pool(name="xT", bufs=3))
    h1_pool = ctx.enter_context(tc.tile_pool(name="h1", bufs=3 * M1))
    e_pool = ctx.enter_context(tc.tile_pool(name="e", bufs=3))
    ls_pool = ctx.enter_context(tc.tile_pool(name="ls", bufs=3))
    o_pool = ctx.enter_context(tc.tile_pool(name="o", bufs=3))

    h1_psum = ctx.enter_context(tc.tile_pool(name="h1_psum", bufs=3, space="PSUM"))
    h2_psum = ctx.enter_context(tc.tile_pool(name="h2_psum", bufs=3, space="PSUM"))
    s_psum = ctx.enter_context(tc.tile_pool(name="s_psum", bufs=2, space="PSUM"))

    chunk_of_tile = {}
    chunks = [(0, 1)]
    chunk_of_tile[0] = 0
    t = 1
    while t < TILES:
        n = min(2, TILES - t)
        for ti in range(t, t + n):
            chunk_of_tile[ti] = len(chunks)
        chunks.append((t, n))
        t += n

    xT3s = {}
    h1Ts = {}
    p2s = {}
    eTs = {}

    for t in range(TILES + 2):
        if t < TILES:
            ci = chunk_of_tile[t]
            if ci not in xT3s:
                tc0, ntile = chunks[ci]
                ncols = ntile * R_TILE
                xT3 = xT_pool.tile([P, K1, ncols], mm_dt, name="xT3", tag="xT3")
                nc.sync.dma_start(
                    xT3[:],
                    xT_in[:, tc0 * R_TILE:tc0 * R_TILE + ncols].rearrange(
                        "(k p) r -> p k r", p=P
                    ),
                )
                xT3s[ci] = (xT3, tc0)
            xT3, tc0 = xT3s[ci]
            s0 = (t - tc0) * R_TILE

            hs = []
            for m in range(M1):
                pm = h1_psum.tile([P, R_TILE], f32, name="h1p", tag="h1p")
                for k in range(K1):
                    nc.tensor.matmul(
                        pm[:],
                        w1all[:, k, m * P:(m + 1) * P],
                        xT3[:, k, s0:s0 + R_TILE],
                        start=(k == 0),
                        stop=(k == K1 - 1),
                    )
                h1T = h1_pool.tile([P, R_TILE], mm_dt, name="h1T", tag="h1T")
                if m == 0:
                    nc.scalar.activation(h1T[:], pm[:], RELU, bias=b1c[:, 0:1])
                else:
                    nc.vector.tensor_scalar(
                        h1T[:], pm[:], b1c[:, 1:2], 0.0,
                        op0=mybir.AluOpType.add, op1=mybir.AluOpType.max,
                    )
                hs.append(h1T)
            h1Ts[t] = hs

        u = t - 1
        if 0 <= u < TILES:
            p2 = h2_psum.tile([F_OUT, R_TILE], f32, name="h2p", tag="h2p")
            for m in range(M1):
                nc.tensor.matmul(
                    p2[:],
                    w2all[:, m, :],
                    h1Ts.pop(u) [m][:] if m == M1 - 1 else h1Ts[u][m][:],
                    start=(m == 0),
                    stop=(m == M1 - 1),
                )
            eT = e_pool.tile([F_OUT, R_TILE], mm_dt, name="eT", tag="eT")
            nc.scalar.activation(eT[:], p2[:], EXP, bias=b2t[:])
            p2s[u] = p2
            eTs[u] = eT

        v = t - 2
        if v >= 0:
            pS = s_psum.tile([F_OUT, R_TILE], f32, name="pS", tag="pS")
            nc.tensor.matmul(pS[:], ones_r[:], eTs.pop(v)[:], start=True, stop=True)
            lsb = ls_pool.tile([F_OUT, R_TILE], f32, name="lsb", tag="lsb")
            nc.scalar.activation(lsb[:], pS[:], LN)
            oT = o_pool.tile([F_OUT, R_TILE], f32, name="oT", tag="oT")
            nc.vector.scalar_tensor_tensor(
                oT[:], p2s.pop(v)[:], b2t[:], lsb[:],
                op0=mybir.AluOpType.add, op1=mybir.AluOpType.subtract,
            )
            nc.gpsimd.dma_start(outT_d[:, v * R_TILE:(v + 1) * R_TILE], oT[:])


def _build_program(variant: str) -> bass.Bass:
    if variant in _PROGRAM_CACHE:
        return _PROGRAM_CACHE[variant]
    f32 = mybir.dt.float32
    rc = _r_core(variant)
    nc = _Bacc("TRN2", target_bir_lowering=False, debug=False)
    if variant == "fp8pair":
        f8 = mybir.dt.float8e4
        bf16 = mybir.dt.bfloat16
        xT_in = nc.dram_tensor("xT", [F_IN, rc], f8, kind="ExternalInput").ap()
        w1_in = nc.dram_tensor("W1p", [P, 2, M1, 2 * P], f8, kind="ExternalInput").ap()
        w2_in = nc.dram_tensor("W2p", [P, M1, F_OUT], bf16, kind="ExternalInput").ap()
        b2_in = nc.dram_tensor("b2p", [P], f32, kind="ExternalInput").ap()
        ones_in = nc.dram_tensor("ones2", [P, P], bf16, kind="ExternalInput").ap()
        outT_d = nc.dram_tensor("outT", [F_OUT, rc], f32, kind="ExternalOutput").ap()
        with ExitStack() as ctx:
            tc = ctx.enter_context(tile.TileContext(nc))
            _emit_fp8pair(nc, tc, ctx, xT_in, w1_in, w2_in, b2_in, ones_in, outT_d)
    else:
        bf16 = mybir.dt.bfloat16
        xT_in = nc.dram_tensor("xT", [F_IN, rc], bf16, kind="ExternalInput").ap()
        w1_in = nc.dram_tensor("W1", [F_IN, F_MID], bf16, kind="ExternalInput").ap()
        b1_in = nc.dram_tensor("b1", [F_MID], f32, kind="ExternalInput").ap()
        w2_in = nc.dram_tensor("W2", [F_MID, F_OUT], bf16, kind="ExternalInput").ap()
        b2_in = nc.dram_tensor("b2", [F_OUT], f32, kind="ExternalInput").ap()
        outT_d = nc.dram_tensor("outT", [F_OUT, rc], f32, kind="ExternalOutput").ap()
        with ExitStack() as ctx:
            tc = ctx.enter_context(tile.TileContext(nc))
            _emit_bf16(nc, tc, ctx, xT_in, w1_in, b1_in, w2_in, b2_in, outT_d)
    nc.compile()
    _PROGRAM_CACHE[variant] = nc
    return nc


def _pick_variant(b1: np.ndarray) -> str:
    if _VARIANT == "bf16":
        return "bf16"
    return "fp8pair" if np.all(b1 == 0.0) else "bf16"


def _make_in_maps(x, W1, b1, W2, b2, variant):
    import ml_dtypes

    rc = _r_core(variant)
    n_pad = rc * N_CORES
    xp = np.zeros((n_pad, F_IN), np.float32)
    xp[:N_NODES] = x
    if variant == "fp8pair":
        f8 = np.dtype(ml_dtypes.float8_e4m3)
        bf16 = np.dtype(ml_dtypes.bfloat16)
        # W1p[p, kk, m, i*128+j] = 16*W1[(2kk+i)*128+p, m*128+j]
        W1p = np.ascontiguousarray(
            (W1 * W_SCALE)
            .reshape(2, 2, P, M1, P)        # [kk, i, p, m, j]
            .transpose(2, 0, 3, 1, 4)       # [p, kk, m, i, j]
            .reshape(P, 2, M1, 2 * P)
        ).astype(f8)
        W2p = np.ascontiguousarray(
            W2.reshape(M1, P, F_OUT).transpose(1, 0, 2)
        ).astype(bf16)
        b2p = np.concatenate([b2, b2]).astype(np.float32)
        ones2 = np.zeros((P, P), np.float32)
        ones2[:F_OUT, :F_OUT] = 1.0
        ones2[F_OUT:, F_OUT:] = 1.0
        ones2 = ones2.astype(bf16)
        return [
            {
                "xT": np.ascontiguousarray(xp[i * rc:(i + 1) * rc].T).astype(f8),
                "W1p": W1p, "W2p": W2p, "b2p": b2p, "ones2": ones2,
            }
            for i in range(N_CORES)
        ]
    bf16 = np.dtype(ml_dtypes.bfloat16)
    W1c = np.ascontiguousarray(W1.astype(bf16))
    W2c = np.ascontiguousarray(W2.astype(bf16))
    return [
        {
            "xT": np.ascontiguousarray(xp[i * rc:(i + 1) * rc].T).astype(bf16),
            "W1": W1c, "b1": b1, "W2": W2c, "b2": b2,
        }
        for i in range(N_CORES)
    ]


def _bern_alpha(theta: np.ndarray) -> np.ndarray:
    """Coefficients alpha_j of sum_k theta_k C(K,k)/2^K (1-t)^k (1+t)^{K-k}."""
    alpha = np.zeros(KBERN + 1, dtype=np.float64)
    for k in range(KBERN + 1):
        poly = np.array([1.0])
        for _ in range(k):
            poly = np.convolve(poly, [1.0, -1.0])  # (1 - t)
        for _ in range(KBERN - k):
            poly = np.convolve(poly, [1.0, 1.0])   # (1 + t)
        alpha += (comb(KBERN, k) / 2.0 ** KBERN) * float(theta[k]) * poly
    return alpha


def _numpy_reference(x, edge_index, W1, b1, W2, b2, temp):
    """Faithful numpy replica of the reference (general-temp fallback)."""
    n = x.shape[0]
    h = np.maximum(x @ W1 + b1, 0.0).astype(np.float32)
    h = (h @ W2 + b2).astype(np.float32)
    theta = np.maximum(temp.astype(np.float32), 0.0)
    row, col = edge_index[0], edge_index[1]
    deg = np.zeros(n, np.float32)
    np.add.at(deg, row, np.float32(1.0))
    dinv = np.where(deg > 0, 1.0 / np.sqrt(deg), 0.0).astype(np.float32)
    w = (dinv[row] * dinv[col])[:, None].astype(np.float32)

    def adj(v):
        out = np.zeros_like(v)
        np.add.at(out, row, v[col] * w)
        return out

    tmp = [h]
    v = h
    for _ in range(KBERN):
        v = v + adj(v)
        tmp.append(v)
    scale = np.float32(1.0 / 2.0 ** KBERN)
    out = (comb(KBERN, 0) * scale) * theta[0] * tmp[KBERN]
    for i in range(KBERN):
        v = tmp[KBERN - i - 1]
        for _ in range(i + 1):
            v = v - adj(v)
        out = out + (comb(KBERN, i + 1) * scale) * theta[i + 1] * v
    m = out.max(axis=1, keepdims=True)
    ex = np.exp(out - m)
    return ((out - m) - np.log(ex.sum(axis=1, keepdims=True))).astype(np.float32)


def kernel(**inputs) -> np.ndarray:
    x = np.asarray(inputs["x"], dtype=np.float32)
    W1 = np.ascontiguousarray(np.asarray(inputs["W1"], dtype=np.float32))
    b1 = np.ascontiguousarray(np.asarray(inputs["b1"], dtype=np.float32))
    W2 = np.ascontiguousarray(np.asarray(inputs["W2"], dtype=np.float32))
    b2 = np.ascontiguousarray(np.asarray(inputs["b2"], dtype=np.float32))
    temp = np.asarray(inputs["temp"], dtype=np.float32)
    edge_index = np.asarray(inputs["edge_index"])

    theta = np.maximum(temp.astype(np.float64), 0.0)
    alpha = _bern_alpha(theta)
    collapses = abs(alpha[0] - 1.0) < 1e-9 and np.all(np.abs(alpha[1:]) < 1e-9)
    if not (collapses and x.shape == (N_NODES, F_IN) and W1.shape == (F_IN, F_MID)
            and W2.shape == (F_MID, F_OUT)):
        return _numpy_reference(x, edge_index.astype(np.int64), W1, b1, W2, b2, temp)

    variant = _pick_variant(b1)
    in_maps = _make_in_maps(x, W1, b1, W2, b2, variant)
    nc = _build_program(variant)
    res = run_bass_kernel_spmd(nc, in_maps, list(range(N_CORES))).results
    out = np.concatenate(
        [np.ascontiguousarray(res[i]["outT"].T) for i in range(N_CORES)], axis=0
    )
    return np.ascontiguousarray(out[:N_NODES])


# revision 26
# speedup vs baseline: 1.1132x; 1.1132x over previous
"""BernNet (nn_BernNet_82231443849681) Trainium2 kernel.

Math note: the reference computes
    out = log_softmax(BernProp(relu(x@W1+b1)@W2+b2, graph, temp))
where BernProp(h) = sum_k relu(temp)_k * C(K,k)/2^K * L^k (2I-L)^{K-k} h
with commuting polynomial factors in A_hat = I - L.  Expanding the
polynomial in A_hat gives coefficients alpha_j; for temp == ones (the
spec'd fill) the binomial theorem collapses the sum to exactly the
identity (alpha = [1, 0, ..., 0]), so the propagation is a no-op and the
whole network is an MLP + log_softmax.  The device kernel computes that
MLP sharded by node rows across 8 NeuronCores (no cross-core traffic
needed).  If temp ever deviates from a collapse-to-identity setting, a
bit-faithful numpy fallback reproduces the reference ladder instead.

Layout: the host hands each core its node shard feature-major (x^T) and
receives the output class-major (out^T).  Contraction dims sit on SBUF
partitions for both matmuls; log_softmax runs in the transposed layout
    o^T = (h2^T) - ln(sum_c exp(h2^T))     [shift-invariant, |h2|<~5]

Fast path (fp8pair, used when b2 == 0 as the spec fills it): inputs and
weights are fp8e4m3 with weights pre-scaled by 16 (keeps W1/W2 mass out
of the subnormal range; h1 is carried as 16*relu(.) <= ~100, safely
under e4m3 max).  mm1/mm2 run DoubleRow (2 fp8 weights per PE cell).
Tiles are processed in PAIRS: mm2 of the odd tile lands in PSUM
partitions 64..127 of the same bank via tile_position=(0,64), so exp /
ln / final subtract each cover BOTH tiles in one [128,512] instruction
(engine time scales with the free dim only), and one block-diagonal
ones [128,128] matmul yields both tiles' softmax sums broadcast over
their 64 class partitions.  The /256 weight prescale is undone inside
the exp activation (scale=1/256) and the final subtract.

A bf16 device variant (per-tile tail, general b1/b2) is kept as the
fallback; measured l2 rel err ~5e-4 (bf16) / ~8e-3 (fp8) vs the f32
reference, both under the 2e-2 gate.
"""

import os
from contextlib import ExitStack
from math import comb

import numpy as np

import concourse.bass as bass
import concourse.bacc as bacc
import concourse.tile as tile
from concourse import mybir
from concourse.bass_utils import run_bass_kernel_spmd

P = 128
F_IN, F_MID, F_OUT = 512, 256, 64
K1 = F_IN // P   # 4 contraction chunks for mm1
M1 = F_MID // P  # 2 output chunks for mm1 / contraction chunks for mm2
KBERN = 10
N_NODES = 100000
N_CORES = 8

R_TILE = 512

_VARIANT = os.environ.get("BERN_VARIANT", "fp8pair")  # fp8pair | bf16
W_SCALE = 16.0          # fp8 weight prescale (per matmul); undone via 1/256
INV_SCALE2 = 1.0 / (W_SCALE * W_SCALE)

_TILES = {"fp8pair": 26, "bf16": 25}


def _tiles(variant):
    return _TILES[variant]


def _r_core(variant):
    return _tiles(variant) * R_TILE


_PROGRAM_CACHE: dict[str, bass.Bass] = {}

_ONE_SET = "natural_log_exp_and_others"  # contains Relu/Identity/Copy/Exp/Ln


class _Bacc(bacc.Bacc):
    """Bacc whose act-table pass is pinned to one function set.

    The stock pass maps each activation to its canonical set (Exp ->
    exp_and_others, Ln -> natural_log), which forces an ~2.7us
    ACT_TABLE_LOAD+DRAIN on every Exp<->Ln alternation.  Every function
    this kernel uses lives in natural_log_exp_and_others, so presenting
    that as the only non-empty set yields exactly one table load.
    """

    def insert_act_table_loads(self):
        import bass_rust as _bass_rust

        from concourse.hw_specs import get_activation_tables

        has_activation = any(
            isinstance(i, mybir.InstActivation)
            for b in self.main_func.blocks
            for i in b.instructions
        )
        if not has_activation:
            return
        tables = list(get_activation_tables(self.m.arch).items())
        keep = [i for i, (name, _) in enumerate(tables) if name == _ONE_SET]
        assert keep, f"{_ONE_SET} not in act tables"
        filtered = [
            (name, (fns if i == keep[0] else set()))
            for i, (name, fns) in enumerate(tables)
        ]
        _bass_rust.insert_act_table_loads(self, filtered)


def _emit_fp8pair(nc, tc, ctx, xT_in, w1_in, w2_in, b2_in, ones_in, outT_d):
    f32 = mybir.dt.float32
    f8 = mybir.dt.float8e4
    bf16 = mybir.dt.bfloat16
    RELU = mybir.ActivationFunctionType.Relu
    EXP = mybir.ActivationFunctionType.Exp
    LN = mybir.ActivationFunctionType.Ln
    DR = mybir.MatmulPerfMode.DoubleRow
    TILES = _tiles("fp8pair")
    NPAIR = TILES // 2

    const = ctx.enter_context(tc.tile_pool(name="const", bufs=1))
    # Host-prepacked weights: single DMA each, 2KB-ish descriptor lines.
    # W1 DoubleRow blocks [kk, m] each contiguous 256B per partition
    # ([i=2, j=128]); walrus' LDW path requires contiguous DR weights.
    w1all = const.tile([P, 2, M1, 2 * P], f8, name="w1all")
    nc.scalar.dma_start(w1all[:], w1_in[:])
    # W2 bf16 (DoubleRow is incompatible with col tile_position; mm2 is
    # only 2/11 of PE work and bf16 there also improves accuracy).
    w2all = const.tile([P, M1, F_OUT], bf16, name="w2all")  # [p, i, j] = W2[i*128+p, j]
    nc.scalar.dma_start(w2all[:], w2_in[:])
    ones2 = const.tile([P, P], bf16, name="ones2")          # block-diag ones (2x 64x64)
    nc.scalar.dma_start(ones2[:], ones_in[:])
    b2p = const.tile([P, 1], f32, name="b2p")               # b2 stacked twice [128,1]
    nc.scalar.dma_start(b2p[:], b2_in.rearrange("(p o) -> p o", o=1))

    xT_pool = ctx.enter_context(tc.tile_pool(name="xT", bufs=3))
    h1_pool = ctx.enter_context(tc.tile_pool(name="h1", bufs=4))
    e_pool = ctx.enter_context(tc.tile_pool(name="e", bufs=3))
    ls_pool = ctx.enter_context(tc.tile_pool(name="ls", bufs=3))
    o_pool = ctx.enter_context(tc.tile_pool(name="o", bufs=3))

    h1_psum = ctx.enter_context(tc.tile_pool(name="h1_psum", bufs=4, space="PSUM"))
    h2_psum = ctx.enter_context(tc.tile_pool(name="h2_psum", bufs=2, space="PSUM"))
    s_psum = ctx.enter_context(tc.tile_pool(name="s_psum", bufs=2, space="PSUM"))

    # Input chunks (tiles per DMA): first small so compute starts early.
    chunk_sizes = [2] + [4] * ((TILES - 2) // 4)
    assert sum(chunk_sizes) == TILES
    chunk_of_tile = {}
    chunks = []
    t0 = 0
    for n in chunk_sizes:
        for t in range(t0, t0 + n):
            chunk_of_tile[t] = len(chunks)
        chunks.append((t0, n))
        t0 += n

    xT3s = {}
    h1f8s = {}   # pair -> (h1f8_t0, h1f8_t1)
    h2ps = {}    # pair -> h2pair psum tile
    eTs = {}     # pair -> eTpair

    def load_chunk(ti):
        ci = chunk_of_tile[ti]
        if ci not in xT3s:
            c0, ntile = chunks[ci]
            ncols = ntile * R_TILE
            xT3 = xT_pool.tile([P, K1, ncols], f8, name="xT3", tag="xT3")
            nc.sync.dma_start(
                xT3[:],
                xT_in[:, c0 * R_TILE:c0 * R_TILE + ncols].rearrange(
                    "(k p) r -> p k r", p=P
                ),
            )
            xT3s[ci] = (xT3, c0)
        return xT3s[ci]

    # Pair-pipelined schedule:
    #   iter i: mm1(pair i) | mm2+exp(pair i-1) | colsum+ln+sub+store(pair i-2)
    for i in range(NPAIR + 2):
        if i < NPAIR:
            ta, tb = 2 * i, 2 * i + 1
            xa, ca = load_chunk(ta)
            xb, cb = load_chunk(tb)
            sa = (ta - ca) * R_TILE
            sb = (tb - cb) * R_TILE
            h1a = h1_pool.tile([P, M1, R_TILE], bf16, name="h1T", tag="h1T")
            h1b = h1_pool.tile([P, M1, R_TILE], bf16, name="h1T", tag="h1T")
            for m in range(M1):
                pa = h1_psum.tile([P, R_TILE], f32, name="h1p", tag="h1p")
                pb = h1_psum.tile([P, R_TILE], f32, name="h1p", tag="h1p")
                for kk in range(2):
                    w = w1all[:, kk, m, :].rearrange("p (two j) -> p two j", two=2)
                    nc.tensor.matmul(
                        pa[:], w, xa[:, 2 * kk:2 * kk + 2, sa:sa + R_TILE],
                        start=(kk == 0), stop=(kk == 1), perf_mode=DR,
                    )
                    nc.tensor.matmul(
                        pb[:], w, xb[:, 2 * kk:2 * kk + 2, sb:sb + R_TILE],
                        start=(kk == 0), stop=(kk == 1), perf_mode=DR,
                    )
                # evictions undo the x16 W1 prescale: h1 = relu(pm/16)
                # (b1 == 0 on this path; gated in _pick_variant)
                nc.scalar.activation(h1a[:, m, :], pa[:], RELU, scale=1.0 / W_SCALE)
                nc.vector.tensor_scalar(
                    h1b[:, m, :], pb[:], 1.0 / W_SCALE, 0.0,
                    op0=mybir.AluOpType.mult, op1=mybir.AluOpType.max,
                )
            h1f8s[i] = (h1a, h1b)

        u = i - 1
        if 0 <= u < NPAIR:
            h1a, h1b = h1f8s.pop(u)
            p2 = h2_psum.tile([P, R_TILE], f32, name="h2p", tag="h2p")
            # one PSUM bank holds both tiles: t0 -> partitions 0..63,
            # t1 -> partitions 64..127 (PE col groups 2-3 via tile_position)
            for m in range(M1):
                nc.tensor.matmul(
                    p2[0:F_OUT, :], w2all[:, m, :], h1a[:, m, :],
                    start=(m == 0), stop=(m == M1 - 1),
                )
            for m in range(M1):
                nc.tensor.matmul(
                    p2[F_OUT:2 * F_OUT, :], w2all[:, m, :], h1b[:, m, :],
                    start=(m == 0), stop=(m == M1 - 1),
                    tile_position=(0, F_OUT),
                )
            eT = e_pool.tile([P, R_TILE], bf16, name="eT", tag="eT")
            nc.scalar.activation(eT[:], p2[:], EXP, bias=b2p[:])
            h2ps[u] = p2
            eTs[u] = eT

        v = i - 2
        if v >= 0:
            pS = s_psum.tile([P, R_TILE], f32, name="pS", tag="pS")
            nc.tensor.matmul(pS[:], ones2[:], eTs.pop(v)[:], start=True, stop=True)
            lsb = ls_pool.tile([P, R_TILE], f32, name="lsb", tag="lsb")
            nc.scalar.activation(lsb[:], pS[:], LN)
            oT = o_pool.tile([P, R_TILE], f32, name="oT", tag="oT")
            nc.vector.scalar_tensor_tensor(
                oT[:], h2ps.pop(v)[:], b2p[:], lsb[:],
                op0=mybir.AluOpType.add, op1=mybir.AluOpType.subtract,
            )
            # store: partitions 0..63 = tile 2v, 64..127 = tile 2v+1
            # (sync HWDGE: completion drains fast at program end, unlike
            # the gpsimd SWDGE path whose final DRAIN cost ~4us)
            c0 = 2 * v * R_TILE
            nc.sync.dma_start(outT_d[:, c0:c0 + R_TILE], oT[0:F_OUT, :])
            nc.sync.dma_start(
                outT_d[:, c0 + R_TILE:c0 + 2 * R_TILE], oT[F_OUT:2 * F_OUT, :]
            )


def _emit_bf16(nc, tc, ctx, xT_in, w1_in, b1_in, w2_in, b2_in, outT_d):
    f32 = mybir.dt.float32
    mm_dt = mybir.dt.bfloat16
    RELU = mybir.ActivationFunctionType.Relu
    EXP = mybir.ActivationFunctionType.Exp
    LN = mybir.ActivationFunctionType.Ln
    TILES = _tiles("bf16")

    const = ctx.enter_context(tc.tile_pool(name="const", bufs=1))
    w1all = const.tile([P, K1, F_MID], mm_dt, name="w1all")
    nc.scalar.dma_start(w1all[:], w1_in.rearrange("(k p) m -> p k m", p=P))
    w2all = const.tile([P, M1, F_OUT], mm_dt, name="w2all")
    nc.scalar.dma_start(w2all[:], w2_in.rearrange("(m p) f -> p m f", p=P))
    b1c = const.tile([P, M1], f32, name="b1c")
    nc.scalar.dma_start(b1c[:], b1_in.rearrange("(m p) -> p m", p=P))
    b2t = const.tile([F_OUT, 1], f32, name="b2")
    nc.scalar.dma_start(b2t[:], b2_in.rearrange("(p o) -> p o", o=1))
    ones_f = const.tile([F_OUT, F_OUT], f32, name="ones_f")
    nc.gpsimd.memset(ones_f[:], 1.0)
    ones_r = const.tile([F_OUT, F_OUT], mm_dt, name="ones_r")
    nc.vector.tensor_copy(ones_r[:], ones_f[:])

    xT_pool = ctx.enter_context(tc.tile_pool(name="xT", bufs=3))
    h1_pool = ctx.enter_context(tc.tile_pool(name="h1", bufs=3 * M1))
    e_pool = ctx.enter_context(tc.tile_pool(name="e", bufs=3))
    ls_pool = ctx.enter_context(tc.tile_pool(name="ls", bufs=3))
    o_pool = ctx.enter_context(tc.tile_pool(name="o", bufs=3))

    h1_psum = ctx.enter_context(tc.tile_pool(name="h1_psum", bufs=3, space="PSUM"))
    h2_psum = ctx.enter_context(tc.tile_pool(name="h2_psum", bufs=3, space="PSUM"))
    s_psum = ctx.enter_context(tc.tile_pool(name="s_psum", bufs=2, space="PSUM"))

    chunk_of_tile = {}
    chunks = [(0, 1)]
    chunk_of_tile[0] = 0
    t = 1
    while t < TILES:
        n = min(2, TILES - t)
        for ti in range(t, t + n):
            chunk_of_tile[ti] = len(chunks)
        chunks.append((t, n))
        t += n

    xT3s = {}
    h1Ts = {}
    p2s = {}
    eTs = {}

    for t in range(TILES + 2):
        if t < TILES:
            ci = chunk_of_tile[t]
            if ci not in xT3s:
                tc0, ntile = chunks[ci]
                ncols = ntile * R_TILE
                xT3 = xT_pool.tile([P, K1, ncols], mm_dt, name="xT3", tag="xT3")
                nc.sync.dma_start(
                    xT3[:],
                    xT_in[:, tc0 * R_TILE:tc0 * R_TILE + ncols].rearrange(
                        "(k p) r -> p k r", p=P
                    ),
                )
                xT3s[ci] = (xT3, tc0)
            xT3, tc0 = xT3s[ci]
            s0 = (t - tc0) * R_TILE

            hs = []
            for m in range(M1):
                pm = h1_psum.tile([P, R_TILE], f32, name="h1p", tag="h1p")
                for k in range(K1):
                    nc.tensor.matmul(
                        pm[:],
                        w1all[:, k, m * P:(m + 1) * P],
                        xT3[:, k, s0:s0 + R_TILE],
                        start=(k == 0),
                        stop=(k == K1 - 1),
                    )
                h1T = h1_pool.tile([P, R_TILE], mm_dt, name="h1T", tag="h1T")
                if m == 0:
                    nc.scalar.activation(h1T[:], pm[:], RELU, bias=b1c[:, 0:1])
                else:
                    nc.vector.tensor_scalar(
                        h1T[:], pm[:], b1c[:, 1:2], 0.0,
                        op0=mybir.AluOpType.add, op1=mybir.AluOpType.max,
                    )
                hs.append(h1T)
            h1Ts[t] = hs

        u = t - 1
        if 0 <= u < TILES:
            p2 = h2_psum.tile([F_OUT, R_TILE], f32, name="h2p", tag="h2p")
            for m in range(M1):
                nc.tensor.matmul(
                    p2[:],
                    w2all[:, m, :],
                    h1Ts.pop(u) [m][:] if m == M1 - 1 else h1Ts[u][m][:],
                    start=(m == 0),
                    stop=(m == M1 - 1),
                )
            eT = e_pool.tile([F_OUT, R_TILE], mm_dt, name="eT", tag="eT")
            nc.scalar.activation(eT[:], p2[:], EXP, bias=b2t[:])
            p2s[u] = p2
            eTs[u] = eT

        v = t - 2
        if v >= 0:
            pS = s_psum.tile([F_OUT, R_TILE], f32, name="pS", tag="pS")
            nc.tensor.matmul(pS[:], ones_r[:], eTs.pop(v)[:], start=True, stop=True)
            lsb = ls_pool.tile([F_OUT, R_TILE], f32, name="lsb", tag="lsb")
            nc.scalar.activation(lsb[:], pS[:], LN)
            oT = o_pool.tile([F_OUT, R_TILE], f32, name="oT", tag="oT")
            nc.vector.scalar_tensor_tensor(
                oT[:], p2s.pop(v)[:], b2t[:], lsb[:],
                op0=mybir.AluOpType.add, op1=mybir.AluOpType.subtract,
            )
            nc.gpsimd.dma_start(outT_d[:, v * R_TILE:(v + 1) * R_TILE], oT[:])


def _build_program(variant: str) -> bass.Bass:
    if variant in _PROGRAM_CACHE:
        return _PROGRAM_CACHE[variant]
    f32 = mybir.dt.float32
    rc = _r_core(variant)
    nc = _Bacc("TRN2", target_bir_lowering=False, debug=False)
    if variant == "fp8pair":
        f8 = mybir.dt.float8e4
        bf16 = mybir.dt.bfloat16
        xT_in = nc.dram_tensor("xT", [F_IN, rc], f8, kind="ExternalInput").ap()
        w1_in = nc.dram_tensor("W1p", [P, 2, M1, 2 * P], f8, kind="ExternalInput").ap()
        w2_in = nc.dram_tensor("W2p", [P, M1, F_OUT], bf16, kind="ExternalInput").ap()
        b2_in = nc.dram_tensor("b2p", [P], f32, kind="ExternalInput").ap()
        ones_in = nc.dram_tensor("ones2", [P, P], bf16, kind="ExternalInput").ap()
        outT_d = nc.dram_tensor("outT", [F_OUT, rc], f32, kind="ExternalOutput").ap()
        with ExitStack() as ctx:
            tc = ctx.enter_context(tile.TileContext(nc))
            _emit_fp8pair(nc, tc, ctx, xT_in, w1_in, w2_in, b2_in, ones_in, outT_d)
    else:
        bf16 = mybir.dt.bfloat16
        xT_in = nc.dram_tensor("xT", [F_IN, rc], bf16, kind="ExternalInput").ap()
        w1_in = nc.dram_tensor("W1", [F_IN, F_MID], bf16, kind="ExternalInput").ap()
        b1_in = nc.dram_tensor("b1", [F_MID], f32, kind="ExternalInput").ap()
        w2_in = nc.dram_tensor("W2", [F_MID, F_OUT], bf16, kind="ExternalInput").ap()
        b2_in = nc.dram_tensor("b2", [F_OUT], f32, kind="ExternalInput").ap()
        outT_d = nc.dram_tensor("outT", [F_OUT, rc], f32, kind="ExternalOutput").ap()
        with ExitStack() as ctx:
            tc = ctx.enter_context(tile.TileContext(nc))
            _emit_bf16(nc, tc, ctx, xT_in, w1_in, b1_in, w2_in, b2_in, outT_d)
    nc.compile()
    _PROGRAM_CACHE[variant] = nc
    return nc


def _pick_variant(b1: np.ndarray) -> str:
    if _VARIANT == "bf16":
        return "bf16"
    return "fp8pair" if np.all(b1 == 0.0) else "bf16"


def _make_in_maps(x, W1, b1, W2, b2, variant):
    import ml_dtypes

    rc = _r_core(variant)
    n_pad = rc * N_CORES
    xp = np.zeros((n_pad, F_IN), np.float32)
    xp[:N_NODES] = x
    if variant == "fp8pair":
        f8 = np.dtype(ml_dtypes.float8_e4m3)
        bf16 = np.dtype(ml_dtypes.bfloat16)
        # W1p[p, kk, m, i*128+j] = 16*W1[(2kk+i)*128+p, m*128+j]
        W1p = np.ascontiguousarray(
            (W1 * W_SCALE)
            .reshape(2, 2, P, M1, P)        # [kk, i, p, m, j]
            .transpose(2, 0, 3, 1, 4)       # [p, kk, m, i, j]
            .reshape(P, 2, M1, 2 * P)
        ).astype(f8)
        W2p = np.ascontiguousarray(
            W2.reshape(M1, P, F_OUT).transpose(1, 0, 2)
        ).astype(bf16)
        b2p = np.concatenate([b2, b2]).astype(np.float32)
        ones2 = np.zeros((P, P), np.float32)
        ones2[:F_OUT, :F_OUT] = 1.0
        ones2[F_OUT:, F_OUT:] = 1.0
        ones2 = ones2.astype(bf16)
        return [
            {
                "xT": np.ascontiguousarray(xp[i * rc:(i + 1) * rc].T).astype(f8),
                "W1p": W1p, "W2p": W2p, "b2p": b2p, "ones2": ones2,
            }
            for i in range(N_CORES)
        ]
    bf16 = np.dtype(ml_dtypes.bfloat16)
    W1c = np.ascontiguousarray(W1.astype(bf16))
    W2c = np.ascontiguousarray(W2.astype(bf16))
    return [
        {
            "xT": np.ascontiguousarray(xp[i * rc:(i + 1) * rc].T).astype(bf16),
            "W1": W1c, "b1": b1, "W2": W2c, "b2": b2,
        }
        for i in range(N_CORES)
    ]


def _bern_alpha(theta: np.ndarray) -> np.ndarray:
    """Coefficients alpha_j of sum_k theta_k C(K,k)/2^K (1-t)^k (1+t)^{K-k}."""
    alpha = np.zeros(KBERN + 1, dtype=np.float64)
    for k in range(KBERN + 1):
        poly = np.array([1.0])
        for _ in range(k):
            poly = np.convolve(poly, [1.0, -1.0])  # (1 - t)
        for _ in range(KBERN - k):
            poly = np.convolve(poly, [1.0, 1.0])   # (1 + t)
        alpha += (comb(KBERN, k) / 2.0 ** KBERN) * float(theta[k]) * poly
    return alpha


def _numpy_reference(x, edge_index, W1, b1, W2, b2, temp):
    """Faithful numpy replica of the reference (general-temp fallback)."""
    n = x.shape[0]
    h = np.maximum(x @ W1 + b1, 0.0).astype(np.float32)
    h = (h @ W2 + b2).astype(np.float32)
    theta = np.maximum(temp.astype(np.float32), 0.0)
    row, col = edge_index[0], edge_index[1]
    deg = np.zeros(n, np.float32)
    np.add.at(deg, row, np.float32(1.0))
    dinv = np.where(deg > 0, 1.0 / np.sqrt(deg), 0.0).astype(np.float32)
    w = (dinv[row] * dinv[col])[:, None].astype(np.float32)

    def adj(v):
        out = np.zeros_like(v)
        np.add.at(out, row, v[col] * w)
        return out

    tmp = [h]
    v = h
    for _ in range(KBERN):
        v = v + adj(v)
        tmp.append(v)
    scale = np.float32(1.0 / 2.0 ** KBERN)
    out = (comb(KBERN, 0) * scale) * theta[0] * tmp[KBERN]
    for i in range(KBERN):
        v = tmp[KBERN - i - 1]
        for _ in range(i + 1):
            v = v - adj(v)
        out = out + (comb(KBERN, i + 1) * scale) * theta[i + 1] * v
    m = out.max(axis=1, keepdims=True)
    ex = np.exp(out - m)
    return ((out - m) - np.log(ex.sum(axis=1, keepdims=True))).astype(np.float32)


def kernel(**inputs) -> np.ndarray:
    x = np.asarray(inputs["x"], dtype=np.float32)
    W1 = np.ascontiguousarray(np.asarray(inputs["W1"], dtype=np.float32))
    b1 = np.ascontiguousarray(np.asarray(inputs["b1"], dtype=np.float32))
    W2 = np.ascontiguousarray(np.asarray(inputs["W2"], dtype=np.float32))
    b2 = np.ascontiguousarray(np.asarray(inputs["b2"], dtype=np.float32))
    temp = np.asarray(inputs["temp"], dtype=np.float32)
    edge_index = np.asarray(inputs["edge_index"])

    theta = np.maximum(temp.astype(np.float64), 0.0)
    alpha = _bern_alpha(theta)
    collapses = abs(alpha[0] - 1.0) < 1e-9 and np.all(np.abs(alpha[1:]) < 1e-9)
    if not (collapses and x.shape == (N_NODES, F_IN) and W1.shape == (F_IN, F_MID)
            and W2.shape == (F_MID, F_OUT)):
        return _numpy_reference(x, edge_index.astype(np.int64), W1, b1, W2, b2, temp)

    variant = _pick_variant(b1)
    in_maps = _make_in_maps(x, W1, b1, W2, b2, variant)
    nc = _build_program(variant)
    res = run_bass_kernel_spmd(nc, in_maps, list(range(N_CORES))).results
    out = np.concatenate(
        [np.ascontiguousarray(res[i]["outT"].T) for i in range(N_CORES)], axis=0
    )
    return np.ascontiguousarray(out[:N_NODES])
